# revision 20
# baseline (speedup 1.0000x reference)
"""Causal self-attention on 8 TRN2 NeuronCores.

Sharding: pure data-parallel on batch (B=8 -> one batch element per core,
no collectives). Each core computes its full [T, C] output slice.

Per-core dataflow (all matmuls bf16 with fp32 PSUM accumulation):
  xT [C,T] (host-pretransposed) --+--> qkT = (w_qk^T @ xT) + b_qk  [2C, T]
                                  +--> V   = (xT^T @ w_v) + b_v    [T, C] (padded
                                       with a ones column per head: [.., 65])
  per head h, per q-chunk (512):
    S^T[k,q] = k_h . q_h           (lhsT = kT_h slice, rhs = qT_h slice)
    E = exp(S^T)                   (ACT, PSUM->SBUF, bf16 out; q pre-scaled
                                    by 1/8 on host so no separate scale op)
    E *= causal mask               (diagonal tiles only; upper tiles skipped)
    O'[d,q], s[q] = [v_h | 1]^T @ E  (augmented-ones row gives softmax sums)
    r = 1/s ; R = ones x r (PE broadcast) ; Y^T block = O' * R
  Z = (Y^T)^T @ w_proj + b_proj    -> DMA PSUM -> DRAM out
Biases enter as K=1 rank-1 accumulate matmuls into the same PSUM group.
"""

import os
import sys
from contextlib import ExitStack

import numpy as np

try:
    import ml_dtypes
except ImportError:  # pragma: no cover
    sys.path.insert(0, "/opt/trn_rl_repo")
    import ml_dtypes

BF16 = ml_dtypes.bfloat16

B, T, C = 8, 1024, 1024
H, HD = 16, 64
N_CORES = 8

# Toggled by test harness to capture a hardware profile.
TRACE = False
LAST_EXEC_NS = None
LAST_RESULTS = None

_NC_CACHE = {}


def _build_nc(Tp, Cp, Hp, HDp, reps=1):
    import concourse.bass as bass
    import concourse.tile as tile
    from concourse import bacc, mybir

    bf = mybir.dt.bfloat16
    f32 = mybir.dt.float32
    AF = mybir.ActivationFunctionType

    P = 128
    CT = Cp // P            # c-tiles (contraction tiles)
    TT = Tp // P            # t-tiles
    QC = min(512, Tp)       # q-chunk width (free dim per matmul)
    NQ = Tp // QC           # q-chunks
    TCH = min(512, Tp)      # t-chunk width for qkT rhs
    TJ = Tp // TCH
    DIAG = QC // P          # diagonal k-tiles per q-chunk
    M2C = 2 * Cp // P       # qk m-chunks
    VJ = Cp // QC           # v/proj column chunks

    nc = bacc.Bacc("TRN2", target_bir_lowering=False, debug=False)

    xT_d = nc.declare_dram_parameter("xT", [Cp, Tp], bf, isOutput=False)
    wqk_d = nc.declare_dram_parameter("w_qk", [Cp, 2 * Cp], bf, isOutput=False)
    wv_d = nc.declare_dram_parameter("w_v", [Cp, Cp], bf, isOutput=False)
    wp_d = nc.declare_dram_parameter("w_proj", [Cp, Cp], bf, isOutput=False)
    bqk_d = nc.declare_dram_parameter("b_qk", [1, 2 * Cp], bf, isOutput=False)
    bv_d = nc.declare_dram_parameter("b_v", [1, Cp], bf, isOutput=False)
    bp_d = nc.declare_dram_parameter("b_proj", [1, Cp], bf, isOutput=False)
    mask_d = nc.declare_dram_parameter("masks", [DIAG, P, QC], bf, isOutput=False)
    out_d = nc.declare_dram_parameter("out", [Tp, Cp], f32, isOutput=True)

    with tile.TileContext(nc) as tc, ExitStack() as ctx:
        consts = ctx.enter_context(tc.tile_pool(name="consts", bufs=1))
        epool = ctx.enter_context(tc.tile_pool(name="epool", bufs=2))
        rpool = ctx.enter_context(tc.tile_pool(name="rpool", bufs=3))
        psum1 = ctx.enter_context(tc.tile_pool(name="psum1", bufs=4, space="PSUM"))
        psum_s = ctx.enter_context(tc.tile_pool(name="psum_s", bufs=2, space="PSUM"))

        # ---- persistent SBUF buffers ----
        xT = consts.tile([P, CT, Tp], bf)
        wqk = consts.tile([P, CT, 2 * Cp], bf)
        wv = consts.tile([P, CT, Cp], bf)
        wp = consts.tile([P, CT, Cp], bf)
        qkT = consts.tile([P, M2C, Tp], bf)
        Vp = consts.tile([P, TT, Hp, HDp + 1], bf)
        YT = consts.tile([P, CT, Tp], bf)
        bqk = consts.tile([1, 2 * Cp], bf)
        bv = consts.tile([1, Cp], bf)
        bp = consts.tile([1, Cp], bf)
        ones = consts.tile([1, max(QC, P)], bf)
        masks = consts.tile([P, DIAG, QC], bf)

        nc.sync.dma_start(xT[:], xT_d.rearrange("(ct p) t -> p ct t", p=P))
        nc.sync.dma_start(wqk[:], wqk_d.rearrange("(ct p) n -> p ct n", p=P))
        nc.sync.dma_start(wv[:], wv_d.rearrange("(ct p) n -> p ct n", p=P))
        nc.sync.dma_start(wp[:], wp_d.rearrange("(ct p) n -> p ct n", p=P))
        nc.sync.dma_start(bqk[:], bqk_d[:])
        nc.sync.dma_start(bv[:], bv_d[:])
        nc.sync.dma_start(bp[:], bp_d[:])
        nc.sync.dma_start(masks[:], mask_d.rearrange("r p q -> p r q"))
        nc.gpsimd.memset(ones[:], 1.0)
        nc.gpsimd.memset(Vp[:], 1.0)  # ones column survives; rest overwritten

        def _emit_qkv():
            # qkT = (w_qk^T @ xT) + b_qk : [2C, T], bf16 copyback
            for m in range(M2C):
                msl = slice(m * P, (m + 1) * P)
                for tj in range(TJ):
                    tsl = slice(tj * TCH, (tj + 1) * TCH)
                    ps = psum1.tile([P, TCH], f32, tag="ps_mm")
                    for ct in range(CT):
                        nc.tensor.matmul(
                            ps[:], lhsT=wqk[:, ct, msl], rhs=xT[:, ct, tsl],
                            start=(ct == 0), stop=False,
                        )
                    nc.tensor.matmul(
                        ps[:], lhsT=bqk[0:1, msl], rhs=ones[0:1, :TCH],
                        start=False, stop=True,
                    )
                    nc.vector.tensor_copy(out=qkT[:, m, tsl], in_=ps[:])
            # V = (x @ w_v) + b_v, interleaved into Vp[.., ti, h, 0:HD]
            for ti in range(TT):
                tsl = slice(ti * P, (ti + 1) * P)
                for vj in range(VJ):
                    vsl = slice(vj * QC, (vj + 1) * QC)
                    ps = psum1.tile([P, QC], f32, tag="ps_mm")
                    for ct in range(CT):
                        nc.tensor.matmul(
                            ps[:], lhsT=xT[:, ct, tsl], rhs=wv[:, ct, vsl],
                            start=(ct == 0), stop=False,
                        )
                    nc.tensor.matmul(
                        ps[:], lhsT=ones[0:1, 0:P], rhs=bv[0:1, vsl],
                        start=False, stop=True,
                    )
                    hpc = QC // HDp  # heads per chunk
                    nc.vector.tensor_copy(
                        out=Vp[:, ti, vj * hpc:(vj + 1) * hpc, 0:HDp],
                        in_=ps[:].rearrange("p (h d) -> p h d", d=HDp),
                    )

        def _emit_attn():
            # qkT m-chunks: [0, M2C/2) = q heads (2 per chunk), rest = k
            for h in range(Hp):
                po = (h % 2) * HDp
                qT = qkT[po:po + HDp, h // 2, :]
                kT = qkT[po:po + HDp, M2C // 2 + h // 2, :]
                for qj in range(NQ):
                    nk = DIAG * (qj + 1)  # active k-tiles (causal)
                    qsl = slice(qj * QC, (qj + 1) * QC)
                    E = epool.tile([P, DIAG * NQ, QC], bf, tag="E")
                    for g in range(nk // 2):
                        ps = psum_s.tile([P, 2, QC], f32, tag="ps_s")
                        for r2 in range(2):
                            ki = 2 * g + r2
                            nc.tensor.matmul(
                                ps[:, r2, :],
                                lhsT=kT[:, ki * P:(ki + 1) * P],
                                rhs=qT[:, qsl],
                                start=True, stop=True,
                            )
                        nc.scalar.activation(E[:, 2 * g:2 * g + 2, :], ps[:], AF.Exp)
                    # causal mask on the DIAG diagonal tiles (the last ones)
                    for rel in range(DIAG):
                        ki = DIAG * qj + rel
                        nc.vector.tensor_mul(
                            out=E[:, ki, :], in0=E[:, ki, :], in1=masks[:, rel, :],
                        )
                    # O'[d,q] + sums row via augmented ones column of Vp
                    pav = psum1.tile([P, QC], f32, tag="ps_mm")
                    for ki in range(nk):
                        nc.tensor.matmul(
                            pav[0:HDp + 1, :],
                            lhsT=Vp[:, ki, h, :], rhs=E[:, ki, :],
                            start=(ki == 0), stop=(ki == nk - 1),
                        )
                    rrow = rpool.tile([1, QC], bf, tag="rrow")
                    with nc.allow_low_precision(
                        reason="bf16 softmax-recip feeds bf16 matmul broadcast"
                    ):
                        nc.vector.reciprocal(rrow[:], pav[HDp:HDp + 1, :])
                    pr = psum1.tile([P, QC], f32, tag="ps_mm")
                    nc.tensor.matmul(
                        pr[0:HDp, :], lhsT=ones[0:1, 0:HDp], rhs=rrow[0:1, :],
                        start=True, stop=True,
                    )
                    rb = rpool.tile([HDp, QC], f32, tag="rb")
                    nc.vector.tensor_copy(out=rb[:], in_=pr[0:HDp, :])
                    nc.vector.tensor_mul(
                        out=YT[po:po + HDp, h // 2, qsl],
                        in0=pav[0:HDp, :], in1=rb[:],
                    )

        def _emit_proj():
            # Z = Y @ w_proj + b_proj -> DRAM
            for ti in range(TT):
                tsl = slice(ti * P, (ti + 1) * P)
                for zj in range(VJ):
                    zsl = slice(zj * QC, (zj + 1) * QC)
                    ps = psum1.tile([P, QC], f32, tag="ps_mm")
                    for ct in range(CT):
                        nc.tensor.matmul(
                            ps[:], lhsT=YT[:, ct, tsl], rhs=wp[:, ct, zsl],
                            start=(ct == 0), stop=False,
                        )
                    nc.tensor.matmul(
                        ps[:], lhsT=ones[0:1, 0:P], rhs=bp[0:1, zsl],
                        start=False, stop=True,
                    )
                    zt = rpool.tile([P, QC], f32, tag="zt")
                    nc.vector.tensor_copy(out=zt[:], in_=ps[:])
                    nc.sync.dma_start(out_d[tsl, zsl], zt[:])

        if reps == 1:
            _emit_qkv()
            _emit_attn()
            _emit_proj()
        else:
            with tc.For_i(0, reps, 1):
                _emit_qkv()
                _emit_attn()
                _emit_proj()

    nc.finalize()
    return nc


def _prep_shared(w_attn, b_attn, w_proj, b_proj):
    """Host-side layout marshalling of the replicated weights (bf16 cast,
    per-head q/k/v column gather, exact 1/8 q pre-scale)."""
    wr = np.asarray(w_attn, np.float32).reshape(C, H, 3, HD)
    w_q = (wr[:, :, 0, :] * np.float32(0.125)).reshape(C, C)
    w_k = wr[:, :, 1, :].reshape(C, C)
    w_qk = np.ascontiguousarray(
        np.concatenate([w_q, w_k], axis=1)
    ).astype(BF16)
    w_v = np.ascontiguousarray(wr[:, :, 2, :].reshape(C, C)).astype(BF16)

    br = np.asarray(b_attn, np.float32).reshape(H, 3, HD)
    b_qk = np.ascontiguousarray(
        np.concatenate(
            [(br[:, 0, :] * np.float32(0.125)).reshape(C), br[:, 1, :].reshape(C)]
        ).reshape(1, 2 * C)
    ).astype(BF16)
    b_v = np.ascontiguousarray(br[:, 2, :].reshape(1, C)).astype(BF16)

    wp = np.ascontiguousarray(np.asarray(w_proj, np.float32)).astype(BF16)
    bp = np.ascontiguousarray(np.asarray(b_proj, np.float32).reshape(1, C)).astype(BF16)

    QCv = min(512, T)
    DIAGv = QCv // 128
    k_idx = np.arange(128)[:, None]
    q_idx = np.arange(QCv)[None, :]
    masks = np.stack(
        [(128 * r + k_idx <= q_idx) for r in range(DIAGv)]
    ).astype(BF16)
    return w_qk, w_v, wp, b_qk, b_v, bp, masks


class _Runner:
    """Cached jit(shard_map) executor for a prebuilt Bass module across
    N cores — same lowering as bass2jax.run_bass_via_pjrt, but reusable
    across calls so warm executions can be timed."""

    def __init__(self, nc, n_cores):
        import jax
        import numpy as _np
        from jax.sharding import Mesh, PartitionSpec
        try:
            from jax.experimental.shard_map import shard_map
        except ImportError:
            from jax.shard_map import shard_map
        from concourse import bass2jax, mybir

        bass2jax.install_neuronx_cc_hook()
        assert not nc.dbg_callbacks
        self.dbg_name = nc.dbg_addr.name if nc.dbg_addr is not None else None
        partition_name = (
            nc.partition_id_tensor.name if nc.partition_id_tensor else None
        )

        in_names, out_names, out_avals = [], [], []
        for alloc in nc.m.functions[0].allocations:
            if not isinstance(alloc, mybir.MemoryLocationSet):
                continue
            name = alloc.memorylocations[0].name
            if alloc.kind == "ExternalInput":
                if name != partition_name:
                    in_names.append(name)
            elif alloc.kind == "ExternalOutput":
                out_names.append(name)
                out_avals.append(
                    jax.core.ShapedArray(
                        tuple(alloc.tensor_shape), mybir.dt.np(alloc.dtype)
                    )
                )
        self.n_params = len(in_names)
        self.in_names = list(in_names)
        self.out_names = out_names
        self.out_avals = out_avals
        self.n_cores = n_cores
        all_names = in_names + out_names
        if partition_name is not None:
            all_names = all_names + [partition_name]

        def _body(*args):
            operands = list(args)
            if partition_name is not None:
                operands.append(bass2jax.partition_id_tensor())
            outs = bass2jax._bass_exec_p.bind(
                *operands,
                out_avals=tuple(out_avals),
                in_names=tuple(all_names),
                out_names=tuple(out_names),
                lowering_input_output_aliases=(),
                sim_require_finite=True,
                sim_require_nnan=True,
                nc=nc,
            )
            return tuple(outs)

        devices = jax.devices()[:n_cores]
        mesh = Mesh(_np.asarray(devices), ("core",))
        n_outs = len(out_names)
        # No donation: the kernel writes every element of every output, so
        # the zero "output seed" operands can live on device and be reused
        # across timed calls.
        self.jitted = jax.jit(
            shard_map(
                _body,
                mesh=mesh,
                in_specs=(PartitionSpec("core"),) * (self.n_params + n_outs),
                out_specs=(PartitionSpec("core"),) * n_outs,
                check_rep=False,
            ),
            keep_unused=True,
        )
        from jax.sharding import NamedSharding

        self.sharding = NamedSharding(mesh, PartitionSpec("core"))
        self.dev_zeros = [
            jax.device_put(
                _np.zeros((n_cores * a.shape[0], *a.shape[1:]), a.dtype),
                self.sharding,
            )
            for a in out_avals
        ]

    def prep_args(self, in_maps):
        import jax
        import numpy as _np

        if self.dbg_name is not None:
            # 8-byte PA as uint32[1,2]; zero -> debugger store+halt skipped
            dbg = _np.zeros((1, 2), _np.uint32)
            in_maps = [{**m, self.dbg_name: dbg} for m in in_maps]
        return [
            jax.device_put(
                _np.concatenate(
                    [_np.asarray(m[name]) for m in in_maps], axis=0
                ),
                self.sharding,
            )
            for name in self.in_names
        ]

    def run(self, concat_in):
        import jax

        out = self.jitted(*concat_in, *self.dev_zeros)
        return jax.block_until_ready(out)

    def results(self, out_arrs):
        import numpy as _np

        return [
            {
                name: _np.asarray(out_arrs[i]).reshape(
                    self.n_cores, *self.out_avals[i].shape
                )[c]
                for i, name in enumerate(self.out_names)
            }
            for c in range(self.n_cores)
        ]


_RUNNER_CACHE = {}


def _get_runner(reps=1):
    key = (T, C, H, HD, reps)
    if key not in _RUNNER_CACHE:
        if key not in _NC_CACHE:
            _NC_CACHE[key] = _build_nc(T, C, H, HD, reps=reps)
        _RUNNER_CACHE[key] = _Runner(_NC_CACHE[key], N_CORES)
    return _RUNNER_CACHE[key]


def _make_in_maps(x, w_attn, b_attn, w_proj, b_proj):
    w_qk, w_v, wp, b_qk, b_v, bp, masks = _prep_shared(
        w_attn, b_attn, w_proj, b_proj
    )
    x = np.asarray(x, np.float32)
    in_maps = []
    for i in range(N_CORES):
        xT = np.ascontiguousarray(x[i].T).astype(BF16)
        in_maps.append({
            "xT": xT, "w_qk": w_qk, "w_v": w_v, "w_proj": wp,
            "b_qk": b_qk, "b_v": b_v, "b_proj": bp, "masks": masks,
        })
    return in_maps


def kernel(x, w_attn, b_attn, w_proj, b_proj):
    runner = _get_runner()
    concat_in = runner.prep_args(
        _make_in_maps(x, w_attn, b_attn, w_proj, b_proj)
    )
    res = runner.results(runner.run(concat_in))
    return np.stack([res[i]["out"] for i in range(N_CORES)]).astype(np.float32)


def measure(x, w_attn, b_attn, w_proj, b_proj, iters=5, reps=1):
    """Warm wall-clock times (s) of the sharded on-device execution.

    reps > 1 uses a kernel variant whose compute body runs `reps` times
    per dispatch (device-side loop), so per-iteration HW time can be
    resolved despite the ~90 ms axon round-trip overhead."""
    import time

    runner = _get_runner(reps=reps)
    concat_in = runner.prep_args(
        _make_in_maps(x, w_attn, b_attn, w_proj, b_proj)
    )
    runner.run(concat_in)  # warm-up / compile
    times = []
    for _ in range(iters):
        t0 = time.perf_counter()
        runner.run(concat_in)
        times.append(time.perf_counter() - t0)
    return times
